# revision 17
# baseline (speedup 1.0000x reference)
"""MinGRU fused kernel for 8 Trainium2 NeuronCores.

Math: the reference's log-space Heinsen scan is the linear recurrence
    h_t = c_t * h_{t-1} + z_t * g(kh_t),   h_0 = g(h0)
with k = x@Wz^T + bz, kh = x@Wh^T + bh, z = sigmoid(k), c = 1 - z,
g(a) = max(a + 0.5, sigmoid(a)) (exact identity for the reference's g).

Device algorithm (per core, batch-sharded 4 batches/core, 2 lane-groups of
2 batches x 256 hidden = 512 lanes):
  - chunked scan, chunk L=128 timesteps on the partition axis
  - lc = ln(1-z); s = cumsum(lc) via triangular matmul (PE)
  - E  = exp(s - m), E2 = exp(m - s) with static shift m = -47
  - w = z*g*E2 ; S' = tri @ w + EM * carry  (carry folded in as a K=1
    accumulating matmul); h = E * S'
All ACT work is phase-blocked by activation-table set (sigmoid vs ln/exp)
to avoid ~2.7us table reloads per switch.
"""

import sys

sys.path.insert(0, "/opt/trn_rl_repo")

import numpy as np

import concourse.bass as bass
import concourse.tile as tile
from concourse import bacc as _bacc
from concourse import mybir
from concourse.bass_utils import run_bass_kernel_spmd

# ---------------- problem constants (hardcoded per task rules) -------------
B, T, DIN, H = 32, 4096, 256, 256
NCORES = 8
BPC = B // NCORES            # batches per core = 4
NGRP = 2                     # lane groups per core
GB = BPC // NGRP             # batches per group = 2
LANES = GB * H               # 512
L = 128                      # chunk length (timesteps per chunk)
NCH = T // L                 # 32 chunks per batch
M_SHIFT = -70.0              # static log-space shift (s spans [0, ~-140])
EM = float(np.exp(np.float32(M_SHIFT)))
BLK = 8                      # chunks per ACT-table phase block (grp-steps = 2*BLK)

F32 = mybir.dt.float32
F32R = mybir.dt.float32r
F16 = mybir.dt.float16
AF = mybir.ActivationFunctionType
OP = mybir.AluOpType

_CACHE = {}


def _build(use_bias: bool, no_carry: bool = False, debug: bool = False):
    nc = bass.Bass()
    xt = nc.dram_tensor("xt", [BPC, DIN, T], F16, kind="ExternalInput")
    wzh = nc.dram_tensor("wzh", [DIN, 2 * H], F16, kind="ExternalInput")
    h0p = nc.dram_tensor("h0p", [L, NGRP * LANES], mybir.dt.bfloat16, kind="ExternalInput")
    em127 = nc.dram_tensor("em127", [L, L], mybir.dt.bfloat16, kind="ExternalInput")
    tris = nc.dram_tensor("tris", [L, L], F16, kind="ExternalInput")
    triw = nc.dram_tensor("triw", [L, L], mybir.dt.bfloat16, kind="ExternalInput")
    bzh = nc.dram_tensor("bzh", [1, 2 * H], F16, kind="ExternalInput")
    out = nc.dram_tensor("out", [NCH, L, BPC, H], F32, kind="ExternalOutput")
    if debug:
        dbg = {
            "dbg_zgs": nc.dram_tensor("dbg_zgs", [NCH, L, 2 * H], F32, kind="ExternalOutput"),
            "dbg_lc": nc.dram_tensor("dbg_lc", [NCH, L, LANES], F16, kind="ExternalOutput"),
            "dbg_s": nc.dram_tensor("dbg_s", [NCH, L, LANES], F32, kind="ExternalOutput"),
            "dbg_e1": nc.dram_tensor("dbg_e1", [NCH, L, LANES], F32, kind="ExternalOutput"),
            "dbg_e2": nc.dram_tensor("dbg_e2", [NCH, L, LANES], F32, kind="ExternalOutput"),
            "dbg_w": nc.dram_tensor("dbg_w", [NCH, L, LANES], mybir.dt.bfloat16, kind="ExternalOutput"),
        }

    from contextlib import ExitStack

    with tile.TileContext(nc) as tc, ExitStack() as ctx:
        const = ctx.enter_context(tc.tile_pool(name="const", bufs=1))
        xpool = ctx.enter_context(tc.tile_pool(name="xpool", bufs=1))
        pA = ctx.enter_context(tc.tile_pool(name="pA", bufs=3))
        pZ = ctx.enter_context(tc.tile_pool(name="pZ", bufs=2 * BLK + 2))
        pB = ctx.enter_context(tc.tile_pool(name="pB", bufs=3))
        pH = ctx.enter_context(tc.tile_pool(name="pH", bufs=4))
        psA = ctx.enter_context(tc.tile_pool(name="psA", bufs=4, space="PSUM"))
        psS = ctx.enter_context(tc.tile_pool(name="psS", bufs=2, space="PSUM"))
        psW = ctx.enter_context(tc.tile_pool(name="psW", bufs=2, space="PSUM"))

        # ---------------- constants -----------------
        w_t = [const.tile([128, 2 * H], F16, tag=f"w{i}", name=f"w{i}") for i in range(2)]
        for i in range(2):
            nc.sync.dma_start(out=w_t[i], in_=wzh[128 * i : 128 * (i + 1), :])
        tris_t = const.tile([L, L], F16, tag="tris", name="tris")
        nc.sync.dma_start(out=tris_t, in_=tris[:, :])
        triw_t = const.tile([L, L], mybir.dt.bfloat16, tag="triw", name="triw")
        nc.sync.dma_start(out=triw_t, in_=triw[:, :])
        bias_m = const.tile([128, 1], F32, tag="bias_m", name="bias_m")
        nc.vector.memset(bias_m, M_SHIFT)
        bias_p = const.tile([128, 1], F32, tag="bias_p", name="bias_p")
        nc.vector.memset(bias_p, -M_SHIFT)
        h0p_t = const.tile([L, NGRP * LANES], mybir.dt.bfloat16, tag="h0p", name="h0p")
        nc.sync.dma_start(out=h0p_t, in_=h0p[:, :])
        em127_t = const.tile([L, L], mybir.dt.bfloat16, tag="em127", name="em127")
        nc.sync.dma_start(out=em127_t, in_=em127[:, :])
        if use_bias:
            bzh_t = const.tile([1, 2 * H], F16, tag="bzh", name="bzh")
            nc.sync.dma_start(out=bzh_t, in_=bzh[:, :])
            ones_t = const.tile([1, 128], F16, tag="ones", name="ones")
            nc.vector.memset(ones_t, 1.0)

        # ---------------- x resident in SBUF -----------------
        x_t = {}
        for b in range(BPC):
            for d in range(2):
                xt_tile = xpool.tile([128, T], F16, tag=f"x{b}_{d}", name=f"x{b}_{d}")
                nc.sync.dma_start(
                    out=xt_tile, in_=xt[b, 128 * d : 128 * (d + 1), :]
                )
                x_t[(b, d)] = xt_tile

        # carry source per group: [128, LANES] f32r AP; row 127 is selected
        # by em127 in the carry matmul
        carry = [h0p_t[:, g * LANES : (g + 1) * LANES] for g in range(NGRP)]

        # ---------------- main loop: blocks of BLK chunks -----------------
        for blk_start in range(0, NCH, BLK):
            chunks = range(blk_start, min(blk_start + BLK, NCH))
            stash = {}
            # ---- phase A: matmuls + sigmoid-set ACT + DVE g/v ----
            for c in chunks:
                for g in range(NGRP):
                    ts_sl = slice(c * L, (c + 1) * L)
                    zgs_b = []
                    vtile = pZ.tile([L, LANES], F32, tag="v", name="v")
                    gtile = pA.tile([L, LANES], F32, tag="g", name="g")
                    for ib in range(GB):
                        b = g * GB + ib
                        kkh = psA.tile([L, 2 * H], F32, tag="kkh", name="kkh")
                        nc.tensor.matmul(
                            kkh, x_t[(b, 0)][:, ts_sl], w_t[0],
                            start=True, stop=False,
                        )
                        nc.tensor.matmul(
                            kkh, x_t[(b, 1)][:, ts_sl], w_t[1],
                            start=False, stop=not use_bias,
                        )
                        if use_bias:
                            nc.tensor.matmul(
                                kkh, ones_t, bzh_t, start=False, stop=True
                            )
                        zgs = pZ.tile([L, 2 * H], F32, tag="zgs", name="zgs")
                        nc.scalar.activation(zgs, kkh, AF.Sigmoid)
                        zgs_b.append(zgs)
                        hsl = slice(ib * H, (ib + 1) * H)
                        # g = max(kh + 0.5, sigmoid(kh))
                        nc.vector.scalar_tensor_tensor(
                            out=gtile[:, hsl], in0=kkh[:, H:], scalar=0.5,
                            in1=zgs[:, H:], op0=OP.add, op1=OP.max,
                        )
                        # v = z * g
                        nc.vector.tensor_tensor(
                            out=vtile[:, hsl], in0=zgs[:, 0:H],
                            in1=gtile[:, hsl], op=OP.mult,
                        )
                    stash[(c, g)] = (zgs_b, vtile)
            # ---- phase B: ln/exp-set ACT + scan matmuls + output ----
            for c in chunks:
                for g in range(NGRP):
                    zgs_b, vtile = stash[(c, g)]
                    lc = pB.tile([L, LANES], F16, tag="lc", name="lc")
                    for ib in range(GB):
                        hsl = slice(ib * H, (ib + 1) * H)
                        # lc = ln(1 - z)
                        nc.scalar.activation(
                            lc[:, hsl], zgs_b[ib][:, 0:H], AF.Ln,
                            bias=1.0, scale=-1.0,
                        )
                    s_ps = psS.tile([L, LANES], F32, tag="s", name="s")
                    nc.tensor.matmul(s_ps, tris_t, lc, start=True, stop=True)
                    e2 = pB.tile([L, LANES], F32, tag="e2", name="e2")
                    nc.scalar.activation(
                        e2, s_ps, AF.Exp, bias=bias_m, scale=-1.0
                    )
                    e1 = pB.tile([L, LANES], F32, tag="e1", name="e1")
                    nc.scalar.activation(
                        e1, s_ps, AF.Exp, bias=bias_p, scale=1.0
                    )
                    w = pB.tile([L, LANES], mybir.dt.bfloat16, tag="w", name="w")
                    nc.vector.tensor_tensor(
                        out=w, in0=vtile, in1=e2, op=OP.mult
                    )
                    sp_ps = psW.tile([L, LANES], F32, tag="S", name="S")
                    nc.tensor.matmul(
                        sp_ps, triw_t, w, start=True, stop=no_carry
                    )
                    if not no_carry:
                        nc.tensor.matmul(
                            sp_ps, em127_t, carry[g], start=False, stop=True
                        )
                    if debug and g == 0:
                        nc.sync.dma_start(out=dbg["dbg_lc"][c], in_=lc)
                        nc.sync.dma_start(out=dbg["dbg_e1"][c], in_=e1)
                        nc.sync.dma_start(out=dbg["dbg_e2"][c], in_=e2)
                        nc.sync.dma_start(out=dbg["dbg_w"][c], in_=w)
                        nc.sync.dma_start(out=dbg["dbg_zgs"][c], in_=stash[(c, g)][0][1])
                        s_dump = pB.tile([L, LANES], F32, tag="s_dump", name="s_dump")
                        nc.vector.tensor_copy(out=s_dump, in_=s_ps)
                        nc.sync.dma_start(out=dbg["dbg_s"][c], in_=s_dump)
                    h = pH.tile([L, LANES], F32, tag="h", name="h")
                    nc.vector.tensor_tensor(
                        out=h, in0=sp_ps, in1=e1, op=OP.mult
                    )
                    hr = pH.tile([L, LANES], mybir.dt.bfloat16, tag="hr", name="hr")
                    nc.gpsimd.tensor_copy(out=hr, in_=h)
                    carry[g] = hr
                    nc.sync.dma_start(
                        out=out[c, :, g * GB : (g + 1) * GB, :],
                        in_=h.rearrange(
                            "p (b hh) -> p b hh", b=GB
                        ),
                    )
    from concourse.bass import _bass_rust as _BR
    _BR.move_matmul_waits_to_ldweights(nc.m)
    _BR.generate_event_semaphores(nc)
    return nc


def _sigmoid(a):
    return np.where(a >= 0, 1.0 / (1.0 + np.exp(-np.abs(a))),
                    np.exp(-np.abs(a)) / (1.0 + np.exp(-np.abs(a))))


def kernel(x, h0, Wz, bz, Wh, bh):
    x = np.asarray(x, np.float32)
    h0 = np.asarray(h0, np.float32)
    Wz = np.asarray(Wz, np.float32)
    bz = np.asarray(bz, np.float32)
    Wh = np.asarray(Wh, np.float32)
    bh = np.asarray(bh, np.float32)

    use_bias = bool(np.any(bz) or np.any(bh))
    key = use_bias
    if key not in _CACHE:
        _CACHE[key] = _build(use_bias)
    nc = _CACHE[key]

    wzh16 = np.ascontiguousarray(
        np.concatenate([Wz.T, Wh.T], axis=1)
    ).astype(np.float16)
    h0f = h0[:, 0, :]  # (B, H)
    h0g = np.where(h0f >= 0, h0f + 0.5, _sigmoid(h0f)).astype(np.float32)
    tri = np.triu(np.ones((L, L), np.float32))
    tris16 = tri.astype(np.float16)
    em127np = np.zeros((L, L), np.float32)
    em127np[L - 1, :] = EM
    import ml_dtypes
    tri_bf = tri.astype(ml_dtypes.bfloat16)
    em127np = em127np.astype(ml_dtypes.bfloat16)
    bzh16 = np.concatenate([bz, bh])[None, :].astype(np.float16)

    in_maps = []
    for i in range(NCORES):
        xc = x[BPC * i : BPC * (i + 1)]                    # (4, T, DIN)
        xt16 = np.ascontiguousarray(
            xc.transpose(0, 2, 1)
        ).astype(np.float16)                                # (4, DIN, T)
        h0c = h0g[BPC * i : BPC * (i + 1)]                  # (4, H)
        import ml_dtypes as _md
        h0pad = np.zeros((L, NGRP * LANES), np.float32)
        h0pad[L - 1, :] = h0c.reshape(-1)
        h0pad = h0pad.astype(_md.bfloat16)
        in_maps.append({
            "xt": xt16,
            "wzh": wzh16,
            "h0p": h0pad,
            "em127": em127np,
            "tris": tris16,
            "triw": tri_bf,
            "bzh": bzh16,
        })

    global _LAST_IN_MAPS
    _LAST_IN_MAPS = in_maps
    res = run_bass_kernel_spmd(nc, in_maps, core_ids=list(range(NCORES)))
    outs = []
    for i in range(NCORES):
        oc = res.results[i]["out"]              # (NCH, L, BPC, H)
        outs.append(
            np.ascontiguousarray(oc.transpose(2, 0, 1, 3)).reshape(BPC, T, H)
        )
    return np.concatenate(outs, axis=0)
